# revision 9
# baseline (speedup 1.0000x reference)
"""CRF loss kernel for nn_CRF_19086834663558 on 8 Trainium2 NeuronCores.

loss = logZ - path_potential, where logZ comes from the forward (alpha)
recursion  v <- (v @ Tm) * E[:, x[t]]  over t = 1..4095, Tm = T[:512].

Parallelization: the per-step transfer matrices M_t = Tm * diag(ext_t) are
strictly positive, so products contract arbitrary start directions onto the
true forward direction at ~40x per step (Birkhoff/Perron-Frobenius; verified
numerically: with 2 warm-up steps the end-to-end rel err is ~1e-6 vs fp64).
The 4095 steps are split into 1024 short segments, each handled by an
independent "chain" that warms up for NW steps on real data (arbitrary
positive seed), then runs its segment.  logZ telescopes exactly over
segments:  logZ = log sum(v_init_chain0) + sum_c [log sum(v_end_c) -
log sum(v_start_c)].

Device layout per core (128 chains): state X = [tag (4x128 partitions),
chain (128 free)], one lockstep iteration =
  16 bf16 matmuls  P[nn] += Tm_tile[kk,nn].T @ X[kk]   (PSUM fp32)
   4 DVE mults     X'[nn] = P[nn] * ext[iter,nn]       (-> bf16 SBUF)
   4 ones-matmuls  colsums of X'                        (PSUM fp32)
No transposes anywhere: the matmul output arrives tag-on-partitions, which
is exactly the layout the next iteration's rhs needs.

Host does the (exact) bits: step 0 / alpha0, E-column gathers for the ext
tiles, the gold-path potential, and the final log assembly in fp64.
"""

import numpy as np
import ml_dtypes

BF16 = ml_dtypes.bfloat16

M_TAGS = 512
L_SEQ = 4096
N_CORES = 8
G = 128                  # chains per core
NC = N_CORES * G         # 1024 chains
NW = 2                   # warm-up steps per chain
S = 4                    # real steps per chain (last chain: 3)
NIT = NW + S             # lockstep device iterations
N_STEPS = L_SEQ - 1      # 4095 recursion steps

_nc_cache = {}


def _build_device_kernel():
    """Bass/Tile kernel: NIT iterations of the batched chain recursion."""
    import concourse.mybir as mybir
    from concourse import bacc
    from concourse.tile import TileContext

    nc = bacc.Bacc(
        "TRN2", target_bir_lowering=False, debug=False, num_devices=N_CORES
    )
    dt = mybir.dt.bfloat16
    # free-dim layouts match SBUF exactly -> one DMA per input
    tm_d = nc.dram_tensor("tm", [128, 16 * 128], dt, kind="ExternalInput")
    ext_d = nc.dram_tensor("ext", [128, NIT * 4 * G], dt, kind="ExternalInput")
    init_d = nc.dram_tensor("init", [128, 4 * G], dt, kind="ExternalInput")
    sums_d = nc.dram_tensor(
        "sums", [1, (NIT + 1) * G], mybir.dt.float32, kind="ExternalOutput"
    )

    with TileContext(nc) as tc:
        with (
            tc.tile_pool(name="const", bufs=1) as cpool,
            tc.tile_pool(name="state", bufs=3) as spool,
            tc.tile_pool(name="pmm", bufs=4, space="PSUM") as ppool,
            tc.tile_pool(name="psum_s", bufs=2, space="PSUM") as pspool,
        ):
            # constants
            tm_sb = cpool.tile([128, 16 * 128], dt)
            nc.sync.dma_start(tm_sb[:], tm_d[:])
            ext_sb = cpool.tile([128, NIT * 4 * G], dt)
            nc.sync.dma_start(ext_sb[:], ext_d[:])
            ones_sb = cpool.tile([128, 1], dt)
            nc.vector.memset(ones_sb[:], 1.0)
            sums_sb = cpool.tile([1, (NIT + 1) * G], mybir.dt.float32)
            # pre-touch ext on DVE: the TT ISA struct supports only one
            # sync-wait, so absorb the ext-DMA wait here instead of on the
            # first P*ext multiply (which must also wait on PE).
            scratch = cpool.tile([128, 1], dt)
            nc.vector.tensor_copy(scratch[:], ext_sb[:, 0:1])

            # initial state
            X = spool.tile([128, 4 * G], dt, tag="state")
            nc.sync.dma_start(X[:], init_d[:])

            def capture_sums(state, j):
                ssum = pspool.tile([1, G], mybir.dt.float32, tag="ssum")
                for kk in range(4):
                    nc.tensor.matmul(
                        ssum[:],
                        ones_sb[:],
                        state[:, kk * G : (kk + 1) * G],
                        start=(kk == 0),
                        stop=(kk == 3),
                    )
                nc.scalar.copy(sums_sb[0:1, j * G : (j + 1) * G], ssum[:])

            capture_sums(X, 0)
            for j in range(1, NIT + 1):
                Xn = spool.tile([128, 4 * G], dt, tag="state")
                for nn in range(4):
                    P = ppool.tile([128, G], mybir.dt.float32, tag="pmm")
                    for kk in range(4):
                        t = (nn * 4 + kk) * 128
                        nc.tensor.matmul(
                            P[:],
                            tm_sb[:, t : t + 128],
                            X[:, kk * G : (kk + 1) * G],
                            start=(kk == 0),
                            stop=(kk == 3),
                        )
                    o = ((j - 1) * 4 + nn) * G
                    nc.vector.tensor_tensor(
                        Xn[:, nn * G : (nn + 1) * G],
                        P[:],
                        ext_sb[:, o : o + G],
                        mybir.AluOpType.mult,
                    )
                capture_sums(Xn, j)
                X = Xn

            nc.sync.dma_start(sums_d[:], sums_sb[:])
    nc.compile()
    return nc


def _get_nc():
    if "nc" not in _nc_cache:
        _nc_cache["nc"] = _build_device_kernel()
    return _nc_cache["nc"]


def _prepare(T, E, Eprev, Enext, Cap, x, y, upper):
    """Host-side prep: returns (in_maps, aux) for the device run."""
    T = np.asarray(T, np.float32)
    E = np.asarray(E, np.float32)
    x = np.asarray(x).astype(np.int64)
    upper = np.asarray(upper).astype(np.int64)

    M = M_TAGS
    B = M
    Tm = np.ascontiguousarray(T[:M])                       # [512, 512]
    Tm_b = Tm.astype(BF16)
    # tm_sb[p, (nn*4+kk)*128 + c] = Tm[kk*128+p, nn*128+c]
    tm_tiles = np.ascontiguousarray(
        Tm_b.reshape(4, 128, 4, 128).transpose(1, 2, 0, 3)
    ).reshape(128, 16 * 128)

    # step 0 (exact, host): alpha0 = exp(phi0)
    phi0 = (
        T[M].astype(np.float64)
        + np.asarray(Eprev, np.float32)[:, B].astype(np.float64)
        + np.asarray(Enext, np.float32)[:, x[1]].astype(np.float64)
        + np.asarray(Cap, np.float32)[:, upper[0]].astype(np.float64)
        + E[:, x[0]].astype(np.float64)
    )
    alpha0 = np.exp(phi0)

    # per-chain step schedule: chain c covers steps [S*c+1, min(S*(c+1),4095)]
    c = np.arange(NC)
    first_step = np.where(c == 0, 1, S * c + 1 - NW)        # iter 1 applies this
    j = np.arange(1, NIT + 1)[:, None]
    step_idx = np.minimum(first_step[None, :] + j - 1, N_STEPS)  # [NIT, NC]

    # ext tiles: ext_full[:, j, c] = bf16(E[:, x[step_idx[j, c]]])
    tok = x[step_idx]                                        # [NIT, NC]
    ext_full = E[:, tok].astype(BF16)                        # [512, NIT, NC]

    # init seeds: chain 0 exact alpha0; others w * ext(prev step)
    w = Tm_b.astype(np.float32).sum(axis=0)                  # [512]
    t_init = np.maximum(S * c - NW, 0)                       # c>=1
    seed = (w[:, None] * E[:, x[t_init]]).astype(BF16)       # [512, NC]
    seed[:, 0] = alpha0.astype(BF16)

    in_maps = []
    for ci in range(N_CORES):
        sl = slice(ci * G, (ci + 1) * G)
        # ext_sb[p, ((j*4+kk))*G + g] = E[kk*128+p, x[step_idx[j, c0+g]]]
        ext_core = np.ascontiguousarray(
            ext_full[:, :, sl].reshape(4, 128, NIT, G).transpose(1, 2, 0, 3)
        ).reshape(128, NIT * 4 * G)
        # init_sb[p, kk*G + g] = seed[kk*128+p, g]
        init_core = np.ascontiguousarray(
            seed[:, sl].reshape(4, 128, G).transpose(1, 0, 2)
        ).reshape(128, 4 * G)
        in_maps.append({"tm": tm_tiles, "ext": ext_core, "init": init_core})

    # capture indices per chain
    seg_end = np.minimum(S * (c + 1), N_STEPS)
    k_start = np.where(c == 0, 0, NW)
    k_end = np.where(c == 0, seg_end, NW + (seg_end - S * c))

    # path potential (exact, host, fp64)
    y_ = np.asarray(y).astype(np.int64)
    y_prev = np.concatenate([[M], y_[:-1]])
    x_prev = np.concatenate([[B], x[:-1]])
    x_next = np.concatenate([x[1:], [B]])
    Ef = E.astype(np.float64)
    phi_path = (
        np.asarray(T, np.float64)[y_prev, y_]
        + np.asarray(Eprev, np.float64)[y_, x_prev]
        + np.asarray(Enext, np.float64)[y_, x_next]
        + np.asarray(Cap, np.float64)[y_, upper]
        + Ef[y_, x]
    ).sum()

    return in_maps, (k_start, k_end, phi_path)


def _assemble(results, aux):
    k_start, k_end, phi_path = aux
    sums = np.concatenate(
        [r["sums"].reshape(NIT + 1, G) for r in results], axis=1
    ).astype(np.float64)                                     # [NIT+1, NC]
    c = np.arange(NC)
    lz = np.log(sums[0, 0])
    lz += (np.log(sums[k_end, c]) - np.log(sums[k_start, c])).sum()
    return np.float32(lz - phi_path)


def _run_device(in_maps, trace=False):
    from concourse.bass_utils import run_bass_kernel_spmd

    nc = _get_nc()
    if trace:
        try:
            return run_bass_kernel_spmd(
                nc, in_maps, core_ids=list(range(N_CORES)), trace=True
            )
        except ModuleNotFoundError:
            pass  # NTFF hook unavailable in this axon build
    return run_bass_kernel_spmd(
        nc, in_maps, core_ids=list(range(N_CORES)), trace=False
    )


def kernel(T, E, Eprev, Enext, Cap, x, y, upper):
    in_maps, aux = _prepare(T, E, Eprev, Enext, Cap, x, y, upper)
    res = _run_device(in_maps, trace=False)
    return _assemble(res.results, aux)


def kernel_traced(T, E, Eprev, Enext, Cap, x, y, upper):
    """Same as kernel() but with NTFF tracing; returns (loss, exec_time_ns,
    BassKernelResults)."""
    in_maps, aux = _prepare(T, E, Eprev, Enext, Cap, x, y, upper)
    res = _run_device(in_maps, trace=True)
    return _assemble(res.results, aux), res.exec_time_ns, res


# revision 13
# speedup vs baseline: 1.5024x; 1.5024x over previous
"""CRF loss kernel for nn_CRF_19086834663558 on 8 Trainium2 NeuronCores.

loss = logZ - path_potential, where logZ comes from the forward (alpha)
recursion  v <- (v @ Tm) * E[:, x[t]]  over t = 1..4095, Tm = T[:512].

Parallelization: the per-step transfer matrices M_t = Tm * diag(ext_t) are
strictly positive, so products contract arbitrary start directions onto the
true forward direction at ~40x per application of Tm (Perron-Frobenius).
The 4095 steps split into 1024 segments of 4 (the last gets 3), each run by
an independent "chain" seeded with colsum(Tm) * ext(previous step) -- one
implicit recursion step -- which after the first real matmul is
indistinguishable from the true direction (verified: end-to-end rel err
~1e-5 vs fp64, tolerance is 2e-2).  logZ telescopes exactly over segments:
  logZ = log sum(seed_0)/scale + sum_c [log sum(end_c) - log sum(start_c)].

Device (per core, 128 chains, lockstep): state X = [tag (4x128 partitions),
chain (128 free)]; one iteration = 16 accumulating 128x128x128 matmuls
(fp8 Tm stationary) + 2 DVE multiplies (PSUM * ext -> bf16 state).  The
matmul output lands tag-on-partitions -- exactly the next iteration's rhs
layout -- so there are no transposes.  Column sums (ones-matmul, captured
at iterations 0/3/4 into separate PSUM banks) DMA straight to HBM at the
end.  Dummy matmuls at kernel start keep the PE HAM clock un-throttled
while the input DMAs stream.

Host does the exact bits: step 0 / alpha0, E-column gathers, the gold-path
potential, and the final log assembly in fp64.
"""

import numpy as np
import ml_dtypes

BF16 = ml_dtypes.bfloat16
FP8 = ml_dtypes.float8_e4m3

M_TAGS = 512
L_SEQ = 4096
N_CORES = 8
G = 128                  # chains per core
NC = N_CORES * G         # 1024 chains
S = 4                    # steps per chain (last chain: 3)
NIT = S                  # lockstep device iterations (no warm-up needed)
N_STEPS = L_SEQ - 1      # 4095 recursion steps
SEED_SCALE = 1.0 / 64.0  # keep seeds in fp8 range; corrected on host
N_WARM_MM = 26           # dummy matmuls to keep PE busy during input DMA
CAPS = (0, 3, 4)         # iterations whose column sums are captured

_nc_cache = {}


def _build_device_kernel():
    import concourse.mybir as mybir
    from concourse import bacc
    from concourse.tile import TileContext

    nc = bacc.Bacc(
        "TRN2", target_bir_lowering=False, debug=False, num_devices=N_CORES
    )
    f8 = mybir.dt.float8e4
    bf = mybir.dt.bfloat16
    f32 = mybir.dt.float32
    tm_d = nc.dram_tensor("tm", [128, 16 * 128], f8, kind="ExternalInput")
    # slot 0 = seeds, slots 1..NIT = per-iteration ext columns
    ext_d = nc.dram_tensor(
        "ext", [128, (NIT + 1) * 4 * G], f8, kind="ExternalInput"
    )
    sums_d = nc.dram_tensor(
        "sums", [1, len(CAPS) * 4 * G], f32, kind="ExternalOutput"
    )

    with TileContext(nc) as tc:
        with (
            tc.tile_pool(name="const", bufs=1) as cpool,
            tc.tile_pool(name="state", bufs=3) as spool,
            tc.tile_pool(name="pmm", bufs=3, space="PSUM") as ppool,
            tc.tile_pool(name="psums", bufs=2, space="PSUM") as pspool,
        ):
            # --- constants / scratch (no DMA deps) ---
            ones_f8 = cpool.tile([128, 1], f8)
            nc.vector.memset(ones_f8[:], 1.0)
            ones_bf = cpool.tile([128, 1], bf)
            nc.vector.memset(ones_bf[:], 1.0)
            warm_sb = cpool.tile([128, 256], bf)
            nc.vector.memset(warm_sb[:], 0.0)

            # --- PE warm-up: junk matmuls while the input DMAs stream.
            # The HAM clock gate needs ~3.4us of sustained PE activity to
            # un-throttle 1.2 -> 2.4 GHz; these also keep the PE from
            # idling into a re-throttle.
            pwarm = pspool.tile([128, 128], f32, tag="pwarm")
            for _ in range(N_WARM_MM):
                nc.tensor.matmul(
                    pwarm[:], warm_sb[:, 0:128], warm_sb[:, 128:256],
                    start=True, stop=True,
                )

            # --- inputs ---
            tm_sb = cpool.tile([128, 16 * 128], f8)
            nc.sync.dma_start(tm_sb[:], tm_d[:])
            ext_sb = cpool.tile([128, (NIT + 1) * 4 * G], f8)
            nc.sync.dma_start(ext_sb[:, 0 : 4 * G], ext_d[:, 0 : 4 * G])
            nc.sync.dma_start(
                ext_sb[:, 4 * G :], ext_d[:, 4 * G :]
            )
            # pre-touch ext on DVE: the TT ISA struct allows one sync-wait;
            # absorb the ext-DMA wait here rather than on the first P*ext
            # multiply (which must also wait on PE).
            scratch = cpool.tile([128, 1], f8)
            nc.vector.tensor_copy(scratch[:], ext_sb[:, (NIT + 1) * 4 * G - 1 :])

            sums_sb = cpool.tile([1, len(CAPS) * 4 * G], f32)

            def capture(state, ones, cap_i):
                sums_ps = pspool.tile([1, 4 * G], f32, tag="ssum")
                nc.tensor.matmul(
                    sums_ps[:], ones[:], state[:], start=True, stop=True
                )
                nc.scalar.copy(
                    sums_sb[0:1, cap_i * 4 * G : (cap_i + 1) * 4 * G],
                    sums_ps[:],
                )

            # capture 0: seeds (fp8, ext slot 0)
            capture(ext_sb[:, 0 : 4 * G], ones_f8, 0)

            X = ext_sb[:, 0 : 4 * G]
            for j in range(1, NIT + 1):
                Xn = spool.tile([128, 4 * G], bf, tag="state")
                for half in range(2):
                    P = ppool.tile([128, 2 * G], f32, tag="pmm")
                    for sub in range(2):
                        nn = 2 * half + sub
                        for kk in range(4):
                            t = (nn * 4 + kk) * 128
                            nc.tensor.matmul(
                                P[:, sub * G : (sub + 1) * G],
                                tm_sb[:, t : t + 128],
                                X[:, kk * G : (kk + 1) * G],
                                start=(kk == 0),
                                stop=(kk == 3),
                            )
                    o = (j * 4 + 2 * half) * G
                    nc.vector.tensor_tensor(
                        Xn[:, half * 2 * G : (half + 1) * 2 * G],
                        P[:],
                        ext_sb[:, o : o + 2 * G],
                        mybir.AluOpType.mult,
                    )
                if j in CAPS:
                    capture(Xn[:], ones_bf, CAPS.index(j))
                X = Xn

            nc.sync.dma_start(sums_d[:], sums_sb[:])
    nc.compile()
    return nc


def _get_nc():
    if "nc" not in _nc_cache:
        _nc_cache["nc"] = _build_device_kernel()
    return _nc_cache["nc"]


def _prepare(T, E, Eprev, Enext, Cap, x, y, upper):
    """Host-side prep: returns (in_maps, aux) for the device run."""
    T = np.asarray(T, np.float32)
    E = np.asarray(E, np.float32)
    x = np.asarray(x).astype(np.int64)
    upper = np.asarray(upper).astype(np.int64)

    M = M_TAGS
    B = M
    Tm = np.ascontiguousarray(T[:M])                       # [512, 512]
    Tm_8 = Tm.astype(FP8)
    # tm_sb[p, (nn*4+kk)*128 + c] = Tm[kk*128+p, nn*128+c]
    tm_tiles = np.ascontiguousarray(
        Tm_8.reshape(4, 128, 4, 128).transpose(1, 2, 0, 3)
    ).reshape(128, 16 * 128)

    # step 0 (exact, host): alpha0 = exp(phi0)
    phi0 = (
        T[M].astype(np.float64)
        + np.asarray(Eprev, np.float32)[:, B].astype(np.float64)
        + np.asarray(Enext, np.float32)[:, x[1]].astype(np.float64)
        + np.asarray(Cap, np.float32)[:, upper[0]].astype(np.float64)
        + E[:, x[0]].astype(np.float64)
    )
    alpha0 = np.exp(phi0)

    # per-chain schedule: chain c covers steps [S*c+1, min(S*(c+1), 4095)];
    # iteration j applies step S*c+j (clamped pad for the short last chain)
    c = np.arange(NC)
    j = np.arange(1, NIT + 1)[:, None]
    step_idx = np.minimum(S * c[None, :] + j, N_STEPS)       # [NIT, NC]
    tok = x[step_idx]                                        # [NIT, NC]
    ext_steps = E[:, tok].astype(FP8)                        # [512, NIT, NC]

    # seeds (slot 0): chain 0 alpha0, others colsum(Tm) * ext(step S*c);
    # all scaled into fp8 range (scale-invariant except chain 0, corrected
    # in _assemble)
    w = Tm_8.astype(np.float32).sum(axis=0)                  # [512]
    seed = (w[:, None] * E[:, x[S * c]]) * SEED_SCALE        # [512, NC]
    seed[:, 0] = alpha0 * SEED_SCALE
    ext_full = np.concatenate(
        [seed.astype(FP8)[:, None, :], ext_steps], axis=1
    )                                                        # [512, NIT+1, NC]

    in_maps = []
    for ci in range(N_CORES):
        sl = slice(ci * G, (ci + 1) * G)
        # ext_sb[p, (j*4+kk)*G + g] = ext_full[kk*128+p, j, c0+g]
        ext_core = np.ascontiguousarray(
            ext_full[:, :, sl].reshape(4, 128, NIT + 1, G).transpose(1, 2, 0, 3)
        ).reshape(128, (NIT + 1) * 4 * G)
        in_maps.append({"tm": tm_tiles, "ext": ext_core})

    # path potential (exact, host, fp64)
    y_ = np.asarray(y).astype(np.int64)
    y_prev = np.concatenate([[M], y_[:-1]])
    x_prev = np.concatenate([[B], x[:-1]])
    x_next = np.concatenate([x[1:], [B]])
    phi_path = (
        np.asarray(T, np.float64)[y_prev, y_]
        + np.asarray(Eprev, np.float64)[y_, x_prev]
        + np.asarray(Enext, np.float64)[y_, x_next]
        + np.asarray(Cap, np.float64)[y_, upper]
        + E.astype(np.float64)[y_, x]
    ).sum()

    return in_maps, phi_path


def _assemble(results, phi_path):
    # sums rows: CAPS index r -> iteration; free layout (kk, g): chain sum
    # = sum over the 4 tag blocks
    sums = np.concatenate(
        [r["sums"].reshape(len(CAPS), 4, G).sum(axis=1) for r in results],
        axis=1,
    ).astype(np.float64)                                     # [len(CAPS), NC]
    s_start = sums[0]                                        # iteration 0
    s_end = sums[2].copy()                                   # iteration 4
    s_end[NC - 1] = sums[1, NC - 1]                          # short last chain
    lz = (np.log(s_end) - np.log(s_start)).sum()
    lz += np.log(s_start[0]) - np.log(SEED_SCALE)
    return np.float32(lz - phi_path)


def _run_device(in_maps, trace=False):
    from concourse.bass_utils import run_bass_kernel_spmd

    nc = _get_nc()
    if trace:
        try:
            return run_bass_kernel_spmd(
                nc, in_maps, core_ids=list(range(N_CORES)), trace=True
            )
        except ModuleNotFoundError:
            pass  # NTFF hook unavailable in this axon build
    return run_bass_kernel_spmd(
        nc, in_maps, core_ids=list(range(N_CORES)), trace=False
    )


def kernel(T, E, Eprev, Enext, Cap, x, y, upper):
    in_maps, phi_path = _prepare(T, E, Eprev, Enext, Cap, x, y, upper)
    res = _run_device(in_maps, trace=False)
    return _assemble(res.results, phi_path)


def kernel_traced(T, E, Eprev, Enext, Cap, x, y, upper):
    """Same as kernel() but requests NTFF tracing; returns (loss,
    exec_time_ns, BassKernelResults)."""
    in_maps, phi_path = _prepare(T, E, Eprev, Enext, Cap, x, y, upper)
    res = _run_device(in_maps, trace=True)
    return _assemble(res.results, phi_path), res.exec_time_ns, res


# revision 18
# speedup vs baseline: 1.8434x; 1.2270x over previous
"""CRF loss kernel for nn_CRF_19086834663558 on 8 Trainium2 NeuronCores.

loss = logZ - path_potential, where logZ comes from the forward (alpha)
recursion  v <- (v @ Tm) * E[:, x[t]]  over t = 1..4095, Tm = T[:512].

Parallelization: the per-step transfer matrices M_t = Tm * diag(ext_t) are
strictly positive, so products contract arbitrary start directions onto
the true forward direction at ~40x per application of Tm
(Perron-Frobenius).  The 4095 steps split into 1024 4-step segments, each
run by an independent "chain" seeded with colsum(Tm) * ext(previous step)
-- one implicit recursion step -- which after the first real matmul is
indistinguishable from the true direction (verified end-to-end rel err
~1.5e-5 vs fp64; tolerance is 2e-2).  Chain 0 owns steps 1..3 plus one
synthetic step whose ext column is alpha0 / (u0 @ Tm), so every chain
runs exactly 4 steps.  logZ then telescopes exactly:

  logZ = sum_c [log sum(end_c) - log sum(seed_c)] + log sum(seed_0)

(seed scales cancel per chain; chain 0 carries the true scale through the
synthetic column).

Device (per core, 128 chains, lockstep): state X = [tag (4x128
partitions), chain (128 free)]; iterations 1..3 are 16 accumulating
128x128x128 fp8 matmuls (Tm tiles stationary) + 2 DVE multiplies
(PSUM * ext -> bf16 state) -- the matmul output lands tag-on-partitions,
exactly the next iteration's rhs layout, so there are no transposes.
Iteration 4 degenerates: only its column sums are needed, and
  sum_n' ext4[n',g] * (Tm^T X3)[n',g]  =  sum_k X3[k,g] * (Tm ext4)[k,g],
so W4 = Tm @ ext4 (16 matmuls with pre-transposed Tm tiles, scheduled
into PE stall windows) and V = X3 * W4 ships to HBM as bf16; the host
does the final sums and logs in fp64.  Dummy matmuls at kernel start run
during the input DMAs to push the PE through its cold-clock (HAM) ramp.

Host does the exact bits: alpha0/step-0, E-column gathers, the gold-path
potential, and the final assembly.
"""

import numpy as np
import ml_dtypes

BF16 = ml_dtypes.bfloat16
FP8 = ml_dtypes.float8_e4m3

M_TAGS = 512
L_SEQ = 4096
N_CORES = 8
G = 128                  # chains per core
NC = N_CORES * G         # 1024 chains
S = 4                    # steps per chain
NIT = 3                  # materialized device iterations (4th degenerates)
N_STEPS = L_SEQ - 1      # 4095 recursion steps
SEED_SCALE = 1.0 / 64.0  # keep seeds in fp8 range (cancels per chain)
N_WARM_MM = 15           # dummy matmuls issued while the input DMAs run

# blob free-dim layout (fp8): tm tiles | seeds | ext slots 1..4 | tmT tiles
OFF_SEED = 16 * 128
OFF_EXT = OFF_SEED + 4 * G
OFF_TMT = OFF_EXT + 4 * 4 * G
W_BLOB = OFF_TMT + 16 * 128

_nc_cache = {}


def _build_device_kernel():
    import concourse.mybir as mybir
    from concourse import bacc
    from concourse.tile import TileContext

    nc = bacc.Bacc(
        "TRN2", target_bir_lowering=False, debug=False, num_devices=N_CORES
    )
    f8 = mybir.dt.float8e4
    bf = mybir.dt.bfloat16
    f32 = mybir.dt.float32
    blob_d = nc.dram_tensor("blob", [128, W_BLOB], f8, kind="ExternalInput")
    v_d = nc.dram_tensor("V", [128, 4 * G], bf, kind="ExternalOutput")

    def ext_off(j):                  # blob offset of ext slot j (j=1..4)
        return OFF_SEED + j * 4 * G

    with TileContext(nc) as tc:
        with (
            tc.tile_pool(name="const", bufs=1) as cpool,
            tc.tile_pool(name="state", bufs=3) as spool,
            tc.tile_pool(name="pmm", bufs=3, space="PSUM") as ppool,
            tc.tile_pool(name="pw", bufs=1, space="PSUM") as pwpool,
        ):
            warm_sb = cpool.tile([128, 128], bf)
            nc.vector.memset(warm_sb[:], 0.0)

            # PE warm-up: junk matmuls while the input DMAs stream; starts
            # the p-state ramp (full clock after ~3us of sustained PE
            # activity).
            pwarm = pwpool.tile([128, 128], f32, tag="pwarm")
            for _ in range(N_WARM_MM):
                nc.tensor.matmul(
                    pwarm[:], warm_sb[:], warm_sb[:], start=True, stop=True
                )

            # input DMAs, sliced in order of first use
            blob_sb = cpool.tile([128, W_BLOB], f8)
            cuts = [0, ext_off(1), ext_off(2), OFF_TMT, W_BLOB]
            for a, b in zip(cuts[:-1], cuts[1:]):
                nc.sync.dma_start(blob_sb[:, a:b], blob_d[:, a:b])
            # pre-touch ext/tmT regions on DVE: the TT ISA struct allows
            # one sync-wait; absorb DMA waits here instead of on the
            # P*ext multiplies (which must also wait on PE).
            scratch = cpool.tile([128, 2], f8)
            nc.vector.tensor_copy(
                scratch[:, 0:1], blob_sb[:, ext_off(2) - 1 : ext_off(2)]
            )
            nc.vector.tensor_copy(
                scratch[:, 1:2], blob_sb[:, OFF_TMT - 1 : OFF_TMT]
            )

            tm_sb = blob_sb[:, 0 : 16 * 128]
            tmT_sb = blob_sb[:, OFF_TMT : OFF_TMT + 16 * 128]
            w4_ps = pwpool.tile([128, 4 * G], f32, tag="w4")

            def w4_unit(kc):
                # W4[k, g] = sum_n' Tm[k, n'] ext4[n', g]; slice kc
                for ncc in range(4):
                    t = (kc * 4 + ncc) * 128
                    nc.tensor.matmul(
                        w4_ps[:, kc * G : (kc + 1) * G],
                        tmT_sb[:, t : t + 128],
                        blob_sb[:, ext_off(4) + ncc * G : ext_off(4) + (ncc + 1) * G],
                        start=(ncc == 0),
                        stop=(ncc == 3),
                    )

            X = blob_sb[:, OFF_SEED : OFF_SEED + 4 * G]
            for j in range(1, NIT + 1):
                Xn = spool.tile([128, 4 * G], bf, tag="state")
                for half in range(2):
                    P = ppool.tile([128, 2 * G], f32, tag="pmm")
                    for sub in range(2):
                        nn = 2 * half + sub
                        for kk in range(4):
                            t = (nn * 4 + kk) * 128
                            nc.tensor.matmul(
                                P[:, sub * G : (sub + 1) * G],
                                tm_sb[:, t : t + 128],
                                X[:, kk * G : (kk + 1) * G],
                                start=(kk == 0),
                                stop=(kk == 3),
                            )
                    o = ext_off(j) + half * 2 * G
                    nc.vector.tensor_tensor(
                        Xn[:, half * 2 * G : (half + 1) * 2 * G],
                        P[:],
                        blob_sb[:, o : o + 2 * G],
                        mybir.AluOpType.mult,
                    )
                X = Xn
                # W4 matmuls fill the PE stall windows after iterations 2/3
                if j == 2:
                    w4_unit(0)
                    w4_unit(1)
                elif j == 3:
                    w4_unit(2)
                    w4_unit(3)

            v_sb = cpool.tile([128, 4 * G], bf)
            nc.vector.tensor_tensor(
                v_sb[:], w4_ps[:], X[:], mybir.AluOpType.mult
            )
            nc.sync.dma_start(v_d[:], v_sb[:])
    nc.compile()
    return nc


def _get_nc():
    if "nc" not in _nc_cache:
        _nc_cache["nc"] = _build_device_kernel()
    return _nc_cache["nc"]


def _pack_tiles(A):
    """[p, (a*4+b)*128 + c] = A[b*128+p, a*128+c] for 4x4 128-tiles."""
    return np.ascontiguousarray(
        A.reshape(4, 128, 4, 128).transpose(1, 2, 0, 3)
    ).reshape(128, 16 * 128)


def _prepare(T, E, Eprev, Enext, Cap, x, y, upper):
    """Host-side prep: returns (in_maps, (seed_sums, phi_path))."""
    T = np.asarray(T, np.float32)
    E = np.asarray(E, np.float32)
    x = np.asarray(x).astype(np.int64)
    upper = np.asarray(upper).astype(np.int64)

    M = M_TAGS
    B = M
    Tm = np.ascontiguousarray(T[:M])                       # [512, 512]
    Tm_8 = Tm.astype(FP8)
    tm_tiles = _pack_tiles(Tm_8)
    tmT_tiles = _pack_tiles(np.ascontiguousarray(Tm_8.T))

    # step 0 (exact, host): alpha0 = exp(phi0)
    phi0 = (
        T[M].astype(np.float64)
        + np.asarray(Eprev, np.float32)[:, B].astype(np.float64)
        + np.asarray(Enext, np.float32)[:, x[1]].astype(np.float64)
        + np.asarray(Cap, np.float32)[:, upper[0]].astype(np.float64)
        + E[:, x[0]].astype(np.float64)
    )
    alpha0 = np.exp(phi0)

    # chain c >= 1 covers steps [4c, 4c+3]; chain 0 covers {syn, 1, 2, 3}.
    # iteration j (1..4) applies step 4c-1+j; chain 0's j=1 column is
    # synthetic: alpha0 / (u0 @ Tm8) with u0 = 1/512.
    c = np.arange(NC)
    j = np.arange(1, S + 1)[:, None]
    step_idx = 4 * c[None, :] - 1 + j                        # [S, NC]
    step_idx[0, 0] = 0                                       # placeholder
    tok = x[step_idx]
    ext_full = E[:, tok]                                     # [512, S, NC] f32
    w8 = Tm_8.astype(np.float32).sum(axis=0)                 # [512] colsum
    syn = (alpha0 * 512.0 / w8.astype(np.float64)).astype(np.float32)
    ext_full[:, 0, 0] = syn
    ext_full = ext_full.astype(FP8)

    # seeds: chain 0 u0 = 1/512 (exact in fp8); others colsum * ext(4c-1)
    seed = (w8[:, None] * E[:, x[4 * c - 1]]) * SEED_SCALE
    seed[:, 0] = 1.0 / 512.0
    seed = seed.astype(FP8)
    seed_sums = seed.astype(np.float64).sum(axis=0)          # [NC] exact

    in_maps = []
    for ci in range(N_CORES):
        sl = slice(ci * G, (ci + 1) * G)
        # slot layout [p, (j*4+kk)*G + g]; slot 0 = seeds
        slots = np.concatenate(
            [seed[:, None, sl], ext_full[:, :, sl]], axis=1
        )                                                    # [512, 5, G]
        slots = np.ascontiguousarray(
            slots.reshape(4, 128, S + 1, G).transpose(1, 2, 0, 3)
        ).reshape(128, (S + 1) * 4 * G)
        blob = np.concatenate(
            [tm_tiles, slots, tmT_tiles], axis=1
        )
        in_maps.append({"blob": blob})

    # path potential (exact, host, fp64)
    y_ = np.asarray(y).astype(np.int64)
    y_prev = np.concatenate([[M], y_[:-1]])
    x_prev = np.concatenate([[B], x[:-1]])
    x_next = np.concatenate([x[1:], [B]])
    phi_path = (
        np.asarray(T, np.float64)[y_prev, y_]
        + np.asarray(Eprev, np.float64)[y_, x_prev]
        + np.asarray(Enext, np.float64)[y_, x_next]
        + np.asarray(Cap, np.float64)[y_, upper]
        + E.astype(np.float64)[y_, x]
    ).sum()

    return in_maps, (seed_sums, phi_path)


def _assemble(results, aux):
    seed_sums, phi_path = aux
    # V layout [p, kk*G + g]: chain sum = sum over partitions and the 4
    # tag blocks
    s_end = np.concatenate(
        [r["V"].astype(np.float64).sum(axis=0).reshape(4, G).sum(axis=0)
         for r in results]
    )                                                        # [NC]
    lz = (np.log(s_end) - np.log(seed_sums)).sum() + np.log(seed_sums[0])
    return np.float32(lz - phi_path)


def _run_device(in_maps, trace=False):
    from concourse.bass_utils import run_bass_kernel_spmd

    nc = _get_nc()
    if trace:
        try:
            return run_bass_kernel_spmd(
                nc, in_maps, core_ids=list(range(N_CORES)), trace=True
            )
        except ModuleNotFoundError:
            pass  # NTFF hook unavailable in this axon build
    return run_bass_kernel_spmd(
        nc, in_maps, core_ids=list(range(N_CORES)), trace=False
    )


def kernel(T, E, Eprev, Enext, Cap, x, y, upper):
    in_maps, aux = _prepare(T, E, Eprev, Enext, Cap, x, y, upper)
    res = _run_device(in_maps, trace=False)
    return _assemble(res.results, aux)


def kernel_traced(T, E, Eprev, Enext, Cap, x, y, upper):
    """Same as kernel() but requests NTFF tracing; returns (loss,
    exec_time_ns, BassKernelResults)."""
    in_maps, aux = _prepare(T, E, Eprev, Enext, Cap, x, y, upper)
    res = _run_device(in_maps, trace=True)
    return _assemble(res.results, aux), res.exec_time_ns, res
